# revision 12
# baseline (speedup 1.0000x reference)
# Trainium2 Bass kernel for nn_ClusterNet (conv1x1 -> 2-layer LSTM -> decoder
# + soft cluster assignment). Pure data parallel over batch: 128 samples ->
# 16 per NeuronCore across 8 cores.
#
# Device-side formulation:
# - Single ACT function (Sigmoid) everywhere: tanh(x) = 2*sigmoid(2x) - 1,
#   with the *2 scalings / -1 offsets folded into host-precomputed weights.
# - LSTM state carried as p = h/2 and c. Layer 2 runs one rotation behind
#   layer 1 (its input y1 = tanh(h1) is produced by an ACT op in between).
# - Gate PSUM tile [64, 32]: rows 0:32 i-units, rows 32:64 f-units at batch
#   cols 0:16; o/g-units at cols 16:32. Unit rows: 0:8 L1, 8:15 L2, rest pad.
# - Engine ops need partition starts % 32 == 0; per-step data rotates along
#   the free dimension (ring buffers) instead of partitions.
# - z and Q staged via DRAM; all outputs written [feature, t, batch] and
#   transposed on the host.
import numpy as np

B, T, F = 128, 8192, 10
F1, H1, H2, K = 8, 8, 7, 16
NC = 8
BL = B // NC          # 16 samples per core
SUB = 32              # epilogue t-block (free dim = SUB*BL = 512)
RU = 8                # u ring slots

_cache = {}


def _prep_consts(w):
    """Host-side constant preparation -> dict of f32 arrays (DRAM inputs)."""
    f32 = np.float32
    w_ih1 = np.asarray(w["w_ih1"], f32)
    w_hh1 = np.asarray(w["w_hh1"], f32)
    b1 = (np.asarray(w["b_ih1"], f32) + np.asarray(w["b_hh1"], f32))
    w_ih2 = np.asarray(w["w_ih2"], f32)
    w_hh2 = np.asarray(w["w_hh2"], f32)
    b2 = (np.asarray(w["b_ih2"], f32) + np.asarray(w["b_hh2"], f32))
    centers = np.asarray(w["centers"], f32)

    # PyTorch gate rows: i, f, g, o
    i1, f1g, g1, o1 = slice(0, 8), slice(8, 16), slice(16, 24), slice(24, 32)
    i2, f2g, g2, o2 = slice(0, 7), slice(7, 14), slice(14, 21), slice(21, 28)

    def mk_p(ga1, ga2, gb1, gb2, sb):
        m = np.zeros((16, 64), f32)
        m[0:8, 0:8] = 2.0 * w_hh1[ga1].T             # h1 = 2*p1
        m[8:15, 8:15] = 2.0 * w_hh2[ga2].T           # h2 = 2*p2
        m[0:8, 32:40] = (2.0 * sb) * w_hh1[gb1].T
        m[8:15, 40:47] = (2.0 * sb) * w_hh2[gb2].T
        return m

    def mk_a(ga1, ga2, gb1, gb2, sb, zero_l2_bias=False):
        m = np.zeros((9, 64), f32)
        m[0:8, 0:8] = w_ih1[ga1].T
        m[0:8, 32:40] = sb * w_ih1[gb1].T
        m[8, 0:8] = b1[ga1]
        m[8, 32:40] = sb * b1[gb1]
        if not zero_l2_bias:
            m[8, 8:15] = b2[ga2] - w_ih2[ga2].sum(1)   # y1 = 2*q1 - 1
            m[8, 40:47] = sb * (b2[gb2] - w_ih2[gb2].sum(1))
        return m

    def mk_q(ga2, gb2, sb):
        m = np.zeros((8, 64), f32)
        m[0:8, 8:15] = 2.0 * w_ih2[ga2].T
        m[0:8, 40:47] = (2.0 * sb) * w_ih2[gb2].T
        return m

    # g-slot rows scaled x2 overall (tanh-via-sigmoid)
    wp_if, wp_og = mk_p(f1g, f2g, i1, i2, 1.0), mk_p(o1, o2, g1, g2, 2.0)
    wa_if, wa_og = mk_a(f1g, f2g, i1, i2, 1.0), mk_a(o1, o2, g1, g2, 2.0)
    wa_if0 = mk_a(f1g, f2g, i1, i2, 1.0, True)
    wa_og0 = mk_a(o1, o2, g1, g2, 2.0, True)
    wq_if, wq_og = mk_q(f2g, i2, 1.0), mk_q(o2, g2, 2.0)

    # conv lhsT [F+1, 9]: col 8 produces a constant-1 row in PSUM (via the
    # ones feature row of padded x) so the bias/ones ride along through the
    # leaky-relu (leaky(1) = 1) into the A buffer row 8.
    wc = np.zeros((F + 1, 9), f32)
    wc[0:F, 0:8] = np.asarray(w["conv_w"], f32).T
    wc[F, 0:8] = np.asarray(w["conv_b"], f32)
    wc[F, 8] = 1.0

    wdec = np.zeros((8, 16), f32)
    wdec[0:7, 0:10] = np.asarray(w["dec_w"], f32)
    wdec[7, 0:10] = np.asarray(w["dec_b"], f32)

    wsim = np.zeros((40, 16), f32)
    wsim[0:7, :] = -2.0 * centers.T
    wsim[7, :] = 1.0 + (centers * centers).sum(1)
    wsim[32:39, :] = 1.0

    ones16 = np.ones((16, 16), f32)
    ones1 = np.ones((1, 1), f32)
    return dict(
        wp_if=wp_if, wp_og=wp_og, wa_if=wa_if, wa_og=wa_og,
        wa_if0=wa_if0, wa_og0=wa_og0, wq_if=wq_if, wq_og=wq_og,
        wc=wc, wdec=wdec, wsim=wsim, ones16=ones16, ones1=ones1,
    )


WSHAPES = {"wp_if": [16, 64], "wp_og": [16, 64], "wa_if": [9, 64],
           "wa_og": [9, 64], "wa_if0": [9, 64], "wa_og0": [9, 64],
           "wq_if": [8, 64], "wq_og": [8, 64], "wc": [F + 1, 9],
           "wdec": [8, 16], "wsim": [40, 16], "ones16": [16, 16],
           "ones1": [1, 1]}


def build(T_, CH_, use_for_i=True, enable_asserts=False, debug_taps=False):
    import concourse.bacc as bacc
    import concourse.bass as bass
    import concourse.mybir as mybir
    import concourse.tile as tile

    NCH_ = T_ // CH_
    assert NCH_ % 2 == 0 and CH_ % RU == 0 and CH_ % 2 == 0
    f32 = mybir.dt.float32
    AF = mybir.ActivationFunctionType
    ALU = mybir.AluOpType

    nc = bacc.Bacc("TRN2", target_bir_lowering=False, debug=False,
                   enable_asserts=enable_asserts)

    xt = nc.dram_tensor("xt", [F + 1, T_ + 2 * CH_, BL], f32, kind="ExternalInput")
    wdram = {n: nc.dram_tensor(n, WSHAPES[n], f32, kind="ExternalInput")
             for n in WSHAPES}

    zq_out = nc.dram_tensor("zq_out", [7, T_ + 1, BL], f32, kind="ExternalOutput")
    xr_out = nc.dram_tensor("xr_out", [10, T_, BL], f32, kind="ExternalOutput")
    fq_out = nc.dram_tensor("fq_out", [K, T_, BL], f32, kind="ExternalOutput")
    fp_out = nc.dram_tensor("fp_out", [K, T_, BL], f32, kind="ExternalOutput")
    q_scr = nc.dram_tensor("q_scr", [K, T_, BL], f32)
    dbg = {}
    if debug_taps:
        for nm, shp in [("dbg_S0", [64, 32]), ("dbg_S1", [64, 32]),
                        ("dbg_U1", [32, BL]), ("dbg_U2", [32, BL]),
                        ("dbg_C1", [32, BL]), ("dbg_A0", [9, CH_ * BL]),
                        ("dbg_Z0", [16, BL]), ("dbg_Z1", [16, BL])]:
            dbg[nm] = nc.dram_tensor(nm, shp, f32, kind="ExternalOutput")

    with tile.TileContext(nc) as tc:
        # persistent ring/state tensors (fixed addresses across For_i iters)
        U = nc.alloc_sbuf_tensor("U", [32, RU * BL], f32)
        CR = nc.alloc_sbuf_tensor("CR", [32, 2 * BL], f32)
        Zq = [nc.alloc_sbuf_tensor(f"Zq{j}", [16, CH_ * BL], f32) for j in range(2)]
        Ab = [nc.alloc_sbuf_tensor(f"A{j}", [9, CH_ * BL], f32) for j in range(2)]
        Eb = [nc.alloc_sbuf_tensor(f"E{j}", [40, SUB * BL], f32) for j in range(2)]
        Csum = nc.alloc_sbuf_tensor("Csum", [16, 16], f32)
        Icolb = nc.alloc_sbuf_tensor("Icolb", [16, SUB * BL], f32)

        with (
            tc.tile_pool(name="const", bufs=1) as constp,
            tc.tile_pool(name="xin", bufs=1) as xinp,
            tc.tile_pool(name="sig", bufs=3) as spool,
            tc.tile_pool(name="small", bufs=2) as smpool,
            tc.tile_pool(name="gpsum", bufs=2, space="PSUM") as gpsum,
            tc.tile_pool(name="cvpsum", bufs=2, space="PSUM") as cvpsum,
            tc.tile_pool(name="episum", bufs=3, space="PSUM") as episum,
            tc.tile_pool(name="epi", bufs=2) as epool,
        ):
            W = {}
            for n in WSHAPES:
                t_ = constp.tile(WSHAPES[n], f32, tag=n)
                nc.sync.dma_start(t_[:], wdram[n].ap())
                W[n] = t_

            nc.vector.memset(U[:], 0.0)
            nc.vector.memset(CR[:], 0.0)
            for z_ in Zq:
                nc.vector.memset(z_[:], 0.0)
            for e_ in Eb:
                nc.vector.memset(e_[:], 0.0)
                # ones row 7 via DMA broadcast (engine writes can't start at 7)
                o1ap = W["ones1"][:]
                nc.sync.dma_start(
                    e_[7:8, :],
                    bass.AP(tensor=o1ap.tensor, offset=o1ap.offset,
                            ap=[[1, 1], [0, SUB * BL], [1, 1]]))
            nc.vector.memset(Csum[:], 0.0)

            def conv_chunk(xoff, abuf):
                xc = xinp.tile([F + 1, CH_ * BL], f32)
                nc.sync.dma_start(xc[:], xt[:, bass.ds(xoff, CH_), :])
                step = min(512, CH_ * BL)
                for s in range(CH_ * BL // step):
                    sl = slice(s * step, (s + 1) * step)
                    ps = cvpsum.tile([9, step], f32)
                    nc.tensor.matmul(ps[:], W["wc"][:], xc[:, sl],
                                     start=True, stop=True)
                    tmp = smpool.tile([9, step], f32)
                    nc.vector.tensor_scalar_mul(tmp[:], ps[:], 0.01)
                    nc.vector.tensor_max(abuf[0:9, sl], ps[:], tmp[:])

            def rotation(rpar, rloc, qsrc, abuf, zchunk, first, flush, gidx=None):
                """rpar: global rotation parity; rloc: position in chunk;
                qsrc: (Zq tensor, slot) or None at r==0; abuf: A source;
                zchunk: Zq tensor receiving sigma(4p) at rloc."""
                su = (rpar % RU) * BL
                su1 = ((rpar + 1) % RU) * BL
                cs = (rpar % 2) * BL
                cs1 = ((rpar + 1) % 2) * BL
                G = gpsum.tile([64, 32], f32)
                wa, wb = ("wa_if0", "wa_og0") if first else ("wa_if", "wa_og")
                arhs = abuf[0:9, rloc * BL:(rloc + 1) * BL]
                if first:
                    qrhs = None
                else:
                    zt, zloc = qsrc
                    qrhs = zt[0:8, zloc * BL:(zloc + 1) * BL]
                # two sequential PSUM accumulation groups (no interleaving)
                nc.tensor.matmul(G[:, 0:16], W["wp_if"][:], U[0:16, su:su + BL],
                                 start=True, stop=False)
                nc.tensor.matmul(G[:, 0:16], W[wa][:], arhs, start=False, stop=first)
                if not first:
                    nc.tensor.matmul(G[:, 0:16], W["wq_if"][:], qrhs,
                                     start=False, stop=True)
                nc.tensor.matmul(G[:, 16:32], W["wp_og"][:], U[0:16, su:su + BL],
                                 start=True, stop=False)
                nc.tensor.matmul(G[:, 16:32], W[wb][:], arhs, start=False, stop=first)
                if not first:
                    nc.tensor.matmul(G[:, 16:32], W["wq_og"][:], qrhs,
                                     start=False, stop=True)
                S = spool.tile([64, 32], f32)
                nc.scalar.activation(S[:], G[:], AF.Sigmoid)
                if debug_taps and gidx in (0, 1):
                    nc.sync.dma_start(dbg[f"dbg_S{gidx}"].ap(), S[:])
                Th = smpool.tile([32, BL], f32)
                nc.vector.scalar_tensor_tensor(Th[:], S[32:64, 16:32], 0.5,
                                               S[32:64, 0:16],
                                               op0=ALU.subtract, op1=ALU.mult)
                P_ = smpool.tile([32, BL], f32)
                nc.vector.tensor_mul(P_[:], S[0:32, 0:16], CR[:, cs:cs + BL])
                nc.vector.scalar_tensor_tensor(CR[:, cs1:cs1 + BL], Th[:], 2.0,
                                               P_[:], op0=ALU.mult, op1=ALU.add)
                Sc = smpool.tile([32, BL], f32)
                nc.scalar.activation(Sc[:], CR[:, cs1:cs1 + BL], AF.Sigmoid,
                                     scale=2.0)
                nc.vector.scalar_tensor_tensor(U[0:32, su1:su1 + BL], Sc[:], 0.5,
                                               S[0:32, 16:32],
                                               op0=ALU.subtract, op1=ALU.mult)
                nc.scalar.activation(zchunk[0:16, rloc * BL:(rloc + 1) * BL],
                                     U[0:16, su1:su1 + BL], AF.Sigmoid, scale=4.0)
                if debug_taps and gidx in (0, 1):
                    nc.sync.dma_start(dbg[f"dbg_U{gidx + 1}"].ap(),
                                      U[:, su1:su1 + BL])
                    nc.sync.dma_start(
                        dbg[f"dbg_Z{gidx}"].ap(),
                        zchunk[0:16, rloc * BL:(rloc + 1) * BL])
                    if gidx == 0:
                        nc.sync.dma_start(dbg["dbg_C1"].ap(),
                                          CR[:, cs1:cs1 + BL])

            def do_chunk(cpar, zdma_off, abuf, has_r0):
                zc, zp = Zq[cpar], Zq[1 - cpar]
                for rl in range(CH_):
                    first = has_r0 and rl == 0
                    qsrc = None if first else (
                        (zp, CH_ - 1) if rl == 0 else (zc, rl - 1))
                    rotation(rl, rl, qsrc, abuf, zc, first, False,
                             gidx=(rl if has_r0 and debug_taps else None))
                nc.sync.dma_start(zq_out[0:7, bass.ds(zdma_off, CH_), :],
                                  zc[8:15, :])

            def pair_body(zoff0, xoff2, xoff3, has_r0):
                do_chunk(0, zoff0, Ab[0], has_r0)
                do_chunk(1, zoff0 + CH_, Ab[1], False)
                conv_chunk(xoff2, Ab[0])
                conv_chunk(xoff3, Ab[1])

            # prologue convs + peeled first pair
            conv_chunk(0, Ab[0])
            conv_chunk(CH_, Ab[1])
            if debug_taps:
                nc.sync.dma_start(dbg["dbg_A0"].ap(), Ab[0][:])
            pair_body(0, 2 * CH_, 3 * CH_, True)

            n_pairs = NCH_ // 2
            if use_for_i and n_pairs > 1:
                with tc.For_i(1, n_pairs, 1) as iv:
                    pair_body(iv * (2 * CH_), iv * (2 * CH_) + 2 * CH_,
                              iv * (2 * CH_) + 3 * CH_, False)
            else:
                for i in range(1, n_pairs):
                    pair_body(i * (2 * CH_), (2 * i + 2) * CH_,
                              (2 * i + 3) * CH_, False)

            # flush rotation r == T_: L1 output garbage (unused), L2 valid.
            fl = NCH_ % 2
            rotation(0, 0, (Zq[1 - fl], CH_ - 1), Ab[0], Zq[fl], False, True)
            nc.sync.dma_start(zq_out[0:7, T_:T_ + 1, :], Zq[fl][8:15, 0:BL])

            # ---------------- epilogue ----------------
            nblocks = T_ // SUB
            for blk in range(nblocks):
                E = Eb[blk % 2]
                t0 = blk * SUB
                nc.sync.dma_start(E[0:7, :], zq_out[0:7, t0 + 1:t0 + 1 + SUB, :])
                nc.vector.tensor_scalar(E[0:7, :], E[0:7, :], 2.0, -1.0,
                                        op0=ALU.mult, op1=ALU.add)
                nc.vector.tensor_mul(E[32:39, :], E[0:7, :], E[0:7, :])
                dps = episum.tile([10, SUB * BL], f32, tag="ep")
                nc.tensor.matmul(dps[:], W["wdec"][:, 0:10], E[0:8, :],
                                 start=True, stop=True)
                xr = epool.tile([10, SUB * BL], f32)
                nc.scalar.copy(xr[:], dps[:])
                nc.sync.dma_start(xr_out[0:10, t0:t0 + SUB, :], xr[:])
                Dps = episum.tile([16, SUB * BL], f32, tag="ep")
                nc.tensor.matmul(Dps[:], W["wsim"][:], E[0:40, :],
                                 start=True, stop=True)
                Qt = epool.tile([16, SUB * BL], f32)
                scr = smpool.tile([16, SUB * BL], f32, tag="scr")
                nc.vector.reciprocal_approx_accurate(Qt[:], Dps[:], scr[:])
                nc.sync.dma_start(q_scr[:, t0:t0 + SUB, :], Qt[:])
                qs = smpool.tile([16, 16], f32)
                nc.vector.tensor_reduce(
                    qs[:], Qt[:].rearrange("k (t b) -> k b t", t=SUB),
                    axis=mybir.AxisListType.X, op=ALU.add)
                nc.vector.tensor_add(Csum[:], Csum[:], qs[:])
                Rps = episum.tile([16, SUB * BL], f32, tag="ep")
                nc.tensor.matmul(Rps[:], W["ones16"][:], Qt[:],
                                 start=True, stop=True)
                Ri = epool.tile([16, SUB * BL], f32)
                scr2 = smpool.tile([16, SUB * BL], f32, tag="scr")
                nc.vector.reciprocal_approx_accurate(Ri[:], Rps[:], scr2[:])
                fqv = epool.tile([16, SUB * BL], f32)
                nc.vector.tensor_mul(fqv[:], Qt[:], Ri[:])
                nc.sync.dma_start(fq_out[:, t0:t0 + SUB, :], fqv[:])

            icol = smpool.tile([16, 16], f32)
            iscr = smpool.tile([16, 16], f32)
            nc.vector.reciprocal_approx_accurate(icol[:], Csum[:], iscr[:])
            icap = icol[:]
            nc.sync.dma_start(
                Icolb[:],
                bass.AP(tensor=icap.tensor, offset=icap.offset,
                        ap=[icap.ap[0], [0, SUB], icap.ap[1]]))
            for blk in range(nblocks):
                t0 = blk * SUB
                Qt = epool.tile([16, SUB * BL], f32)
                nc.sync.dma_start(Qt[:], q_scr[:, t0:t0 + SUB, :])
                Pv = epool.tile([16, SUB * BL], f32)
                nc.vector.tensor_mul(Pv[:], Qt[:], Qt[:])
                nc.vector.tensor_mul(Pv[:], Pv[:], Icolb[:])
                Rps = episum.tile([16, SUB * BL], f32, tag="ep")
                nc.tensor.matmul(Rps[:], W["ones16"][:], Pv[:],
                                 start=True, stop=True)
                Ri = epool.tile([16, SUB * BL], f32)
                scr3 = smpool.tile([16, SUB * BL], f32, tag="scr")
                nc.vector.reciprocal_approx_accurate(Ri[:], Rps[:], scr3[:])
                fpv = epool.tile([16, SUB * BL], f32)
                nc.vector.tensor_mul(fpv[:], Pv[:], Ri[:])
                nc.sync.dma_start(fp_out[:, t0:t0 + SUB, :], fpv[:])

    nc.finalize()
    return nc


def make_in_maps(x, consts, T_, CH_):
    in_maps = []
    for c in range(NC):
        xc = np.asarray(x[c * BL:(c + 1) * BL], np.float32)   # [BL, T_, F]
        xtc = np.zeros((F + 1, T_ + 2 * CH_, BL), np.float32)
        xtc[0:F, 0:T_] = xc.transpose(2, 1, 0)
        xtc[F, :] = 1.0
        m = {"xt": np.ascontiguousarray(xtc)}
        m.update(consts)
        in_maps.append(m)
    return in_maps


def collect_outputs(results, T_):
    zs, xrs, fqs, fps = [], [], [], []
    for c in range(NC):
        r = results[c]
        zq = r["zq_out"]
        zs.append((2.0 * zq[:, 1:T_ + 1, :] - 1.0).transpose(2, 1, 0))
        xrs.append(r["xr_out"].transpose(2, 1, 0))
        fqs.append(r["fq_out"].transpose(2, 1, 0))
        fps.append(r["fp_out"].transpose(2, 1, 0))
    z = np.ascontiguousarray(np.concatenate(zs, 0), np.float32)
    xr = np.ascontiguousarray(np.concatenate(xrs, 0), np.float32)
    fq = np.ascontiguousarray(np.concatenate(fqs, 0), np.float32)
    fp = np.ascontiguousarray(np.concatenate(fps, 0), np.float32)
    return z, xr, fp, fq


_last_exec_ns = None


def kernel(**inputs):
    import os
    import time

    from concourse.bass_utils import run_bass_kernel_spmd

    global _last_exec_ns
    x = np.asarray(inputs["x"], np.float32)
    consts = _prep_consts(inputs)
    CH_ = 512
    key = ("nc", T, CH_)
    if key not in _cache:
        _cache[key] = build(T, CH_, use_for_i=True)
    nc = _cache[key]
    in_maps = make_in_maps(x, consts, T, CH_)
    trace = bool(int(os.environ.get("KERNEL_TRACE", "0")))
    t0 = time.time()
    res = run_bass_kernel_spmd(nc, in_maps, core_ids=list(range(NC)),
                               trace=trace)
    wall_ns = int((time.time() - t0) * 1e9)
    _last_exec_ns = res.exec_time_ns if res.exec_time_ns else wall_ns
    return collect_outputs(res.results, T)
